# revision 34
# baseline (speedup 1.0000x reference)
"""GPT-2 style multi-head attention on 8 Trainium2 cores (Bass/Tile).

Problem: B=2, T=2048, C=1024, H=16 heads, D=64, fp32 in/out.

Sharding (hardcoded): 2 groups x 4 cores; group g handles batch b=g.
Within a group, rank r computes heads [4r, 4r+4) (tensor parallel over
heads: c_attn column slices), then AllGather of y^T across the group,
then each core computes a 256-column slice of the output projection
(c_proj column slice) plus bias.

All matmul operands are bf16 (PSUM accumulation stays fp32).  fp32(r)
matmuls on TRN2 disable fast-weight-load, serialize ~200ns LDWEIGHTS
per matmul into the PE timeline, and draw enough power that the
HAM/GPIO throttlers pin the PE at 1.2 GHz; bf16 avoids all three.
x is transposed on the HOST and shipped as bf16 x^T [C, T]: the kernel
needs only x^T (lhsT for V, rhs for qk^T), which removes all 128 PE
transposes and halves the x DMA bytes.

Schedule notes (from NTFF traces of earlier revisions):
 - The PE executes in program order, so anything the PE waits on
   (softmax-denominator reciprocal on DVE: 3.3us for a single-partition
   [1,512] row) stalls every later matmul.  The normalize is therefore
   split: the reciprocal runs on ACT as ln -> exp(-x) (both read row 64
   directly, ~0.7us each, and share one activation table set), and the
   PE-side broadcast matmul + DVE multiply are deferred until after the
   NEXT t-block's stage-1 matmuls have been emitted, by which point the
   reciprocal is long done.
 - AV matmuls are emitted one k-tile behind their QK pair so the ACT
   exp of tile kt hides under the QK matmuls of tile kt+1.
 - The AllGather has a ~27us fixed cost regardless of payload; all four
   output projections are deferred to the tail so proj(0..2) hides most
   of AllGather(3).
 - PSUM: scores/stage1/proj accumulators share one [P,2,512] tag
   (2 banks x 2 bufs) and the four per-head y_aug tiles get their own
   bank each (4 x 1), totalling exactly 8 banks.

Per-core dataflow, pipelined over 512-row t-blocks:
  stage 1: qk^T[., tb] = W_qk^T @ x^T (per-partition bias on the PSUM
           copyback, bf16 out); V[tb] = x @ W_v (bias via ones-row
           matmul into the accumulating PSUM), stored per 128-row
           k-tile with an appended ones column so the AV matmul also
           emits the softmax denominator for free.
  stage 2 (q block qb=tb): per head pair (even/odd heads on partition
           halves of the same qkT chunk): both heads' scores into one
           2-bank PSUM tile -> one exp(0.125*s) on ACT -> one
           broadcast-masked multiply (diagonal tiles only) -> per head
           y_aug^T[65,512] += V_aug^T @ e^T (row 64 = sum of exp).
  stage 3: AllGather y^T slice (bf16) across the 4-core group, then
           out[:, col slice] = y^T.T @ W_p slice + bias.
"""

import numpy as np

import concourse.bass as bass  # noqa: F401  (bass types via tc)
import concourse.mybir as mybir
import concourse.tile as tile
from concourse import bacc

P = 128
B, T_FULL, C, H, D = 2, 2048, 1024, 16, 64
F32 = mybir.dt.float32
BF16 = mybir.dt.bfloat16
EXP = mybir.ActivationFunctionType.Exp
LN = mybir.ActivationFunctionType.Ln
BYPASS = mybir.AluOpType.bypass


class Cfg:
    def __init__(self, n_cores, group_size, T, repeat=1, **_compat):
        self.repeat = repeat
        self.n_cores = n_cores
        self.GS = group_size               # cores per batch group
        self.T = T                         # sequence length per core
        self.HL = H // group_size          # heads per core
        assert self.HL % 2 == 0
        self.NP = C // group_size          # c_proj columns per core
        self.CC = C // P                   # contraction chunks (8)
        self.TB = T // 512                 # t-blocks == q blocks
        self.QB = T // 512
        self.KT = T // P                   # k tiles
        self.QKCH = self.HL                # qk^T partition chunks (Q | K)
        self.VW = 68                       # per-head V stride: 64 V + 1 ones
        if n_cores == 8:
            self.replica_groups = [[0, 1, 2, 3], [4, 5, 6, 7]]
        elif n_cores == 4:
            self.replica_groups = [[0, 1], [2, 3]]
        elif n_cores == 1:
            self.replica_groups = [[0]]
        else:
            raise ValueError(n_cores)


CFG_FULL = Cfg(8, 4, T_FULL)


def _patch_act_tables():
    """Make Exp and Ln resolve to the one table set containing both.

    bacc's insert_act_table_loads picks, per activation, the first
    act_info.json set containing its function: Exp -> exp_and_others,
    Ln -> natural_log.  A kernel using both then reloads tables
    (~1.3us each, on ACT) at every normalize.  Restricting Exp/Ln to
    natural_log_exp_and_others (set indices preserved) yields a single
    table load for the whole kernel.
    """
    import concourse.bacc as bacc_mod
    import concourse.hw_specs as hw_specs
    if getattr(bacc_mod, "_mha_act_tables_patched", False):
        return
    orig = hw_specs.get_activation_tables
    keep = "natural_log_exp_and_others"

    def patched(arch):
        t = {k: set(v) for k, v in orig(arch).items()}
        if keep in t and EXP in t[keep] and LN in t[keep]:
            for name in t:
                if name != keep:
                    t[name] -= {EXP, LN}
        return t

    bacc_mod.get_activation_tables = patched
    bacc_mod._mha_act_tables_patched = True


def emit(tc, outs, ins, cfg):
    """Emit the SPMD program. outs/ins are dicts of DRAM APs."""
    for rep in range(cfg.repeat):
        _emit_once(tc, outs["out"], ins, cfg, rep)


def _emit_once(tc, out, ins, cfg, rep):
    nc = tc.nc
    GS, T, HL, NP, CC, VW = cfg.GS, cfg.T, cfg.HL, cfg.NP, cfg.CC, cfg.VW
    QKCH = cfg.QKCH

    xT = ins["xT"]            # [P, CC, T] bf16 (host-transposed x, SBUF layout)
    wqk = ins["wqk"]          # [P, QKCH, CC, 128] bf16 (SBUF layout, m-major)
    wv = ins["wv"]            # [P, CC, HL*64] bf16 (SBUF layout)
    bqk = ins["bqk"]          # [P, QKCH] f32 (chunk-major per-partition bias)
    wp = ins["wp"]            # [P, CC, NP] bf16 (SBUF layout)
    bp = ins["bp"]            # [1, NP] bf16 (c_proj bias + folded V bias)
    masks = ins["masks"]      # [P, 4, 512] bf16

    from contextlib import ExitStack
    with ExitStack() as _stk:
        persist = _stk.enter_context(tc.tile_pool(name="persist", bufs=1))
        s2 = _stk.enter_context(tc.tile_pool(name="s2", bufs=4))
        s3 = _stk.enter_context(tc.tile_pool(name="s3", bufs=2))
        dram = _stk.enter_context(
            tc.tile_pool(name="dram", bufs=1, space="DRAM"))
        # PSUM: "s" [P,2,512] x2 = 4 banks shared by scores + all f32
        # accumulators; "y" [65,512] x4 = 4 banks.  Total 8.
        ps = _stk.enter_context(tc.tile_pool(
            name="ps", bufs=2, space="PSUM"))
        ps_y = _stk.enter_context(tc.tile_pool(
            name="ps_y", bufs=4, space="PSUM"))

        def acc_tile(name):
            # 1-bank accumulator carved out of a 2-bank "s" slot
            t = ps.tile([P, 2, 512], F32, tag="s", name=name)
            return t[:, 0, :]

        # ---- persistent SBUF tensors ----
        xT_all = persist.tile([P, CC, T], BF16, tag="xT")
        qkT = persist.tile([P, QKCH, T], BF16, tag="qkT")
        vsb = persist.tile([P, cfg.KT, HL * VW], BF16, tag="vsb")
        mask_sb = persist.tile([P, 4, 512], BF16, tag="mask")
        ones_row = persist.tile([1, P], BF16, tag="ones_row")
        ones65 = persist.tile([65, 64], BF16, tag="ones65")
        wp_sb = persist.tile([P, CC, NP], BF16, tag="wp")
        bp_sb = persist.tile([1, NP], BF16, tag="bp")
        wqk_sb = persist.tile([P, QKCH, CC, P], BF16, tag="wqk")
        wv_sb = persist.tile([P, CC, HL * D], BF16, tag="wv")
        bqk_sb = persist.tile([P, QKCH], F32, tag="bqk")
        ag_sb = [persist.tile([P, CC, 512], BF16, tag=f"ag{qb}",
                              name=f"ag_sb{qb}_{rep}")
                 for qb in range(cfg.QB)]

        # All weight tensors arrive in their SBUF layout (contiguous
        # per-partition lines; see make_core_inputs).  Only the bytes
        # stage1(0) needs go first — x^T block 0 per cc-chunk on the
        # sync ring (subtile deps let the cc-loop chase the DMA) racing
        # just wqk/bqk on the gpsimd ring; everything else is ordered
        # behind them on the same two rings by need time (masks at
        # attention(0) ~25us, wv mid-stage1, x^T rest at stage1(1)).
        for cc in range(CC):
            nc.sync.dma_start(xT_all[:, cc, 0:512], xT[:, cc, 0:512])
        nc.gpsimd.dma_start(wqk_sb[:, 0], wqk[:, 0])
        nc.gpsimd.dma_start(bqk_sb[:], bqk)
        for m in range(1, QKCH):
            nc.gpsimd.dma_start(wqk_sb[:, m], wqk[:, m])
        nc.sync.dma_start(mask_sb[:], masks)
        nc.gpsimd.dma_start(wv_sb[:], wv)
        nc.sync.dma_start(xT_all[:, :, 512:], xT[:, :, 512:])
        nc.gpsimd.dma_start(wp_sb[:], wp)
        nc.gpsimd.dma_start(bp_sb[:], bp)

        nc.vector.memset(ones_row[:], 1.0)
        nc.vector.memset(ones65[64:65, :], 1.0)
        vsb_h = vsb.rearrange("p k (h w) -> p k h w", w=VW)
        nc.vector.memset(vsb_h[:, :, :, 64:65], 1.0)

        ag_in = [
            dram.tile([HL * D, 512], BF16, tag=f"agin{qb}",
                      name=f"agin{qb}_{rep}")
            for qb in range(cfg.QB)
        ]
        ag_out = [
            dram.tile([GS * HL * D, 512], BF16, tag=f"agout{qb}",
                      name=f"agout{qb}_{rep}")
            for qb in range(cfg.QB)
        ]


        def stage1_qk(tb):
            tsl = slice(tb * 512, (tb + 1) * 512)
            # qk^T: lhsT = W chunk (stationary), rhs = x^T chunk
            for m in range(QKCH):
                acc = acc_tile(f"acc{tb}_{m}_{rep}")
                for cc in range(CC):
                    nc.tensor.matmul(
                        acc,
                        wqk_sb[:, m, cc, :],
                        xT_all[:, cc, tsl],
                        start=(cc == 0),
                        stop=(cc == CC - 1),
                    )
                nc.vector.tensor_scalar_add(
                    qkT[:, m, tsl], acc, bqk_sb[:, m:m + 1])

        def stage1_v(tb):
            # V natural: lhsT = x^T chunk (stationary), rhs = W_v.
            # No bias matmul: bv is constant per (head, d), so it passes
            # through the softmax average unchanged and is folded into
            # the c_proj bias on the host.
            for ts in range(4):
                kt = tb * 4 + ts
                c0 = tb * 512 + ts * P
                vp = acc_tile(f"vp{kt}_{rep}")
                for cc in range(CC):
                    nc.tensor.matmul(
                        vp[:, 0:HL * D],
                        xT_all[:, cc, c0:c0 + P],
                        wv_sb[:, cc, :],
                        start=(cc == 0),
                        stop=(cc == CC - 1),
                    )
                nc.vector.tensor_copy(
                    vsb_h[:, kt, :, 0:64],
                    vp[:, 0:HL * D].rearrange("p (h d) -> p h d", d=D),
                )

        def norm_pre(qb, h, y):
            # softmax denominator reciprocal on ACT: rec = exp(-ln d).
            # Both funcs live in the natural_log_exp table set, and ACT
            # reads the PSUM row directly — no single-partition DVE op.
            lnd = s2.tile([65, 512], F32, tag="lnd", bufs=2,
                          name=f"lnd{qb}_{h}_{rep}")
            nc.scalar.activation(lnd[64:65, :], y[64:65, :], LN)
            rec = s2.tile([65, 512], BF16, tag="rec", bufs=4,
                          name=f"rec{qb}_{h}_{rep}")
            nc.scalar.activation(rec[64:65, :], lnd[64:65, :], EXP,
                                 scale=-1.0)
            return rec

        def norm_post(qb, h, y, rec):
            bc = acc_tile(f"bc{qb}_{h}_{rep}")
            nc.tensor.matmul(
                bc[0:64, :], ones65[64:65, :], rec[64:65, :],
                start=True, stop=True,
            )
            bc_sb = s2.tile([64, 512], F32, tag="bc_sb", bufs=2,
                            name=f"bcs{qb}_{h}_{rep}")
            nc.vector.tensor_copy(bc_sb[:], bc[0:64, :])
            yn = s2.tile([64, 512], BF16, tag="yn", bufs=4,
                         name=f"yn{qb}_{h}_{rep}")
            nc.vector.tensor_mul(yn[:], y[0:64, :], bc_sb[:])
            nc.sync.dma_start(ag_in[qb][h * 64:(h + 1) * 64, :], yn[:])

        def _kt_order(qb):
            # one unmasked off-diagonal tile first (its AV has no DVE
            # mask dependency, and the flush/stage1 DVE backlog at the
            # block boundary hasn't drained yet), then the diagonal
            # tiles (longest exp->mask->AV chains), then the rest.
            nkt = 4 * qb + 4
            return (list(range(min(1, qb))) +
                    list(range(4 * qb, nkt)) +
                    list(range(min(1, qb), 4 * qb)))

        def attention_pairs(qb, hps):
            # The head pairs' kt loops interleaved: the PE alternates
            # pairs' QK matmuls (each pair's even/odd heads overlap on
            # disjoint row strips), AV pairs are emitted one k-tile
            # behind, so every exp hides under independent QK work.
            nkt = 4 * qb + 4
            kt_order = _kt_order(qb)
            pbs = [slice(0, 64), slice(64, 128)]
            ys = {hp: [ps_y.tile([65, 512], F32, tag="y",
                                 name=f"y{qb}_{2 * hp + i}_{rep}")
                       for i in range(2)]
                  for hp in hps}
            # AV emission runs AV_DEPTH k-tiles behind QK: the first
            # AVs land ~2.5us into the block, past both the exp latency
            # and the boundary DVE backlog (yn muls releasing y banks).
            AV_DEPTH = 2
            av_pend = []
            for ki, kt in enumerate(kt_order):
                j = kt - 4 * qb
                lo = 128 * j if j > 0 else 0
                es = []
                for hp in hps:
                    qch, kch = hp, QKCH // 2 + hp
                    s = ps.tile([P, 2, 512], F32, tag="s",
                                name=f"s{qb}_{kt}_{hp}_{rep}")
                    for i in range(2):
                        nc.tensor.matmul(
                            s[:, i, lo:],
                            qkT[pbs[i], kch, kt * P:(kt + 1) * P],
                            qkT[pbs[i], qch, qb * 512 + lo:(qb + 1) * 512],
                            start=True, stop=True,
                        )
                    es.append(s)
                if len(av_pend) >= AV_DEPTH:
                    for prev in av_pend.pop(0):
                        prev()
                av_prev = []
                for hx, hp in enumerate(hps):
                    e = s2.tile([P, 2, 512], BF16, tag="e", bufs=6,
                                name=f"e{qb}_{kt}_{hp}_{rep}")
                    nc.scalar.activation(
                        e[:, :, lo:], es[hx][:, :, lo:], EXP, scale=0.125)
                    if j >= 0:
                        nc.vector.tensor_mul(
                            e[:, :, lo:], e[:, :, lo:],
                            mask_sb[:, j:j + 1, lo:].to_broadcast(
                                [P, 2, 512 - lo]))

                    def av(ki=ki, lo=lo, kt=kt, e=e, hp=hp):
                        for i in range(2):
                            nc.tensor.matmul(
                                ys[hp][i][:, lo:],
                                vsb[:, kt,
                                    (2 * hp + i) * VW:(2 * hp + i) * VW + 65],
                                e[:, i, lo:],
                                start=(ki == 0), stop=(ki == nkt - 1),
                            )
                    av_prev.append(av)
                av_pend.append(av_prev)
            for group in av_pend:
                for prev in group:
                    prev()
            posts = []
            for hp in hps:
                for i in range(2):
                    h = 2 * hp + i
                    posts.append((qb, h, ys[hp][i],
                                  norm_pre(qb, h, ys[hp][i])))
            return posts

        def attention(qb):
            return attention_pairs(qb, list(range(HL // 2)))

        def flush(posts):
            for qb, h, y, rec in posts:
                norm_post(qb, h, y, rec)

        def allgather(qb):
            nc.gpsimd.collective_compute(
                "AllGather", BYPASS,
                replica_groups=cfg.replica_groups,
                ins=[ag_in[qb].opt()],
                outs=[ag_out[qb].opt()],
            )
            # prefetch the gathered y^T into SBUF on the gpsimd ring:
            # its wait-for-collective would stall any other engine's
            # pipeline (a DMA_DIRECT2D sem-wait blocks the issuing
            # engine), but gpsimd only runs the collectives, which
            # serialize on the CC cores anyway.  The last block is on
            # the critical tail: slice it per tt so proj(3)'s first
            # matmuls chase the DMA.
            ag_r = ag_out[qb].rearrange("(c p) t -> p c t", p=P)
            if qb < cfg.QB - 1:
                nc.gpsimd.dma_start(ag_sb[qb][:], ag_r[:])
            else:
                for tt in range(4):
                    tsl = slice(tt * P, (tt + 1) * P)
                    nc.gpsimd.dma_start(ag_sb[qb][:, :, tsl],
                                        ag_r[:, :, tsl])

        def proj(qb):
            for tt in range(4):
                tsl = slice(tt * P, (tt + 1) * P)
                op = acc_tile(f"op{qb}_{tt}_{rep}")
                for cc in range(CC):
                    nc.tensor.matmul(
                        op[:, 0:NP], ag_sb[qb][:, cc, tsl],
                        wp_sb[:, cc, :], start=(cc == 0), stop=False,
                    )
                nc.tensor.matmul(
                    op[:, 0:NP], ones_row[:1, :], bp_sb[:1, :],
                    start=False, stop=True,
                )
                # PSUM -> SBUF copyback on ACT (idle at the tail, and
                # closer to PSUM than DVE), Copy is in every table set
                o_sb = s3.tile([P, NP], F32, tag="osb", bufs=4)
                nc.scalar.activation(
                    o_sb[:], op[:, 0:NP],
                    mybir.ActivationFunctionType.Copy)
                row = (qb * 4 + tt) * P
                nc.sync.dma_start(out[row:row + P, :], o_sb[:])

        # pipeline: attention(qb) consumes exactly the k-tiles stage1(tb)
        # produced; normalize posts flush between the next stage1's QK
        # and V parts — late enough that the ACT reciprocal chain is
        # done (no PE wait on the bc matmuls), early enough that the
        # yn multiplies clear DVE before the next attention's first AV
        # recycles the y banks.  Projs all run in the tail where
        # proj(0..2) hides AllGather(3)'s fixed cost.
        stage1_qk(0)
        stage1_v(0)
        for tb in range(cfg.TB):
            posts = attention(tb)
            if tb + 1 < cfg.TB:
                stage1_qk(tb + 1)
                flush(posts)
                allgather(tb)
                stage1_v(tb + 1)
            else:
                flush(posts)
                allgather(tb)
        for qb in range(cfg.QB):
            proj(qb)


def make_core_inputs(x_full, c_attn_w, c_attn_b, c_proj_w, c_proj_b, cfg,
                     core):
    """Host-side input sharding (and bf16 cast) for one core."""
    import ml_dtypes
    bf16 = ml_dtypes.bfloat16

    GS, HL, NP, T = cfg.GS, cfg.HL, cfg.NP, cfg.T
    g, rk = divmod(core, GS)
    g = g % B  # tolerate more groups than batches (sim configs)
    hs = slice(rk * HL * D, (rk + 1) * HL * D)
    wq = c_attn_w[:, 0 * C:1 * C][:, hs]
    wk = c_attn_w[:, 1 * C:2 * C][:, hs]
    wv = c_attn_w[:, 2 * C:3 * C][:, hs]
    bq = c_attn_b[0 * C:1 * C][hs]
    bk = c_attn_b[1 * C:2 * C][hs]
    bv = c_attn_b[2 * C:3 * C][hs]
    cs = slice(rk * NP, (rk + 1) * NP)

    pp = np.arange(P)[:, None, None]
    jj = np.arange(4)[None, :, None]
    qq = np.arange(512)[None, None, :]
    masks = (qq >= pp + 128 * jj)

    CCn = cfg.CC

    def sbuf_layout(w):
        # [C, N] -> [P, CC, N]: partition-major chunked contraction
        return np.ascontiguousarray(
            w.reshape(CCn, P, w.shape[1]).transpose(1, 0, 2))

    wqk_cn = np.concatenate([wq, wk], axis=1).astype(bf16)  # [C, QKCH*128]
    # [C, M*128] -> [P, M, CC, 128] (m-major so each m-slice is one
    # contiguous 2KB-per-partition DMA)
    wqk_pre = np.ascontiguousarray(
        wqk_cn.reshape(CCn, P, cfg.QKCH, P).transpose(1, 2, 0, 3))

    # bv is constant per (head, d): it passes through the softmax
    # average unchanged, so its contribution to the output is exactly
    # bv_full @ Wp — fold it into the c_proj bias here.
    bv_full = c_attn_b[2 * C:3 * C]
    bp_folded = bv_full @ c_proj_w[:, cs] + c_proj_b[cs]

    return {
        "xT": sbuf_layout(x_full[g, :T].T.astype(bf16)),
        "wqk": wqk_pre,
        "wv": sbuf_layout(wv.astype(bf16)),
        "bqk": np.ascontiguousarray(
            np.concatenate([bq, bk]).reshape(cfg.QKCH, P).T, np.float32),
        "wp": sbuf_layout(c_proj_w[:, cs].astype(bf16)),
        "bp": np.ascontiguousarray(bp_folded[None, :].astype(bf16)),
        "masks": masks.astype(bf16),
    }


_CACHE = {}


def _input_shapes_dtypes(cfg):
    return {
        "xT": ((P, cfg.CC, cfg.T), BF16),
        "wqk": ((P, cfg.QKCH, cfg.CC, P), BF16),
        "wv": ((P, cfg.CC, cfg.HL * D), BF16),
        "bqk": ((P, cfg.QKCH), F32),
        "wp": ((P, cfg.CC, cfg.NP), BF16),
        "bp": ((1, cfg.NP), BF16),
        "masks": ((P, 4, 512), BF16),
    }


def _build_full():
    if "nc" in _CACHE:
        return _CACHE["nc"]
    _patch_act_tables()
    cfg = CFG_FULL
    nc = bacc.Bacc(
        "TRN2", target_bir_lowering=False, debug=False,
        num_devices=cfg.n_cores,
    )
    ins = {}
    for name, (shape, dt) in _input_shapes_dtypes(cfg).items():
        ins[name] = nc.dram_tensor(
            name, list(shape), dt, kind="ExternalInput").ap()
    outs = {
        "out": nc.dram_tensor(
            "out", [cfg.T, cfg.NP], F32, kind="ExternalOutput").ap()
    }
    with tile.TileContext(nc) as tc:
        emit(tc, outs, ins, cfg)
    nc.compile()
    _CACHE["nc"] = nc
    return nc


def kernel(**inputs):
    from concourse.bass_utils import run_bass_kernel_spmd

    cfg = CFG_FULL
    x = np.asarray(inputs["x"], np.float32)
    c_attn_w = np.asarray(inputs["c_attn_w"], np.float32)
    c_attn_b = np.asarray(inputs["c_attn_b"], np.float32)
    c_proj_w = np.asarray(inputs["c_proj_w"], np.float32)
    c_proj_b = np.asarray(inputs["c_proj_b"], np.float32)

    nc = _build_full()
    in_maps = [
        make_core_inputs(x, c_attn_w, c_attn_b, c_proj_w, c_proj_b, cfg, core)
        for core in range(cfg.n_cores)
    ]
    res = run_bass_kernel_spmd(nc, in_maps, core_ids=list(range(cfg.n_cores)))
    out = np.empty((B, T_FULL, C), np.float32)
    for core in range(cfg.n_cores):
        g, rk = divmod(core, cfg.GS)
        out[g, :, rk * cfg.NP:(rk + 1) * cfg.NP] = res.results[core]["out"]
    return out


# revision 38
# speedup vs baseline: 1.0475x; 1.0475x over previous
"""GPT-2 style multi-head attention on 8 Trainium2 cores (Bass/Tile).

Problem: B=2, T=2048, C=1024, H=16 heads, D=64, fp32 in/out.

Sharding (hardcoded): 2 groups x 4 cores; group g handles batch b=g.
Within a group, rank r computes heads [4r, 4r+4) (tensor parallel over
heads: c_attn column slices), then AllGather of y^T across the group,
then each core computes a 256-column slice of the output projection
(c_proj column slice) plus bias.

All matmul operands are bf16 (PSUM accumulation stays fp32).  fp32(r)
matmuls on TRN2 disable fast-weight-load, serialize ~200ns LDWEIGHTS
per matmul into the PE timeline, and draw enough power that the
HAM/GPIO throttlers pin the PE at 1.2 GHz; bf16 avoids all three.
x is transposed on the HOST and shipped as bf16 x^T [C, T]: the kernel
needs only x^T (lhsT for V, rhs for qk^T), which removes all 128 PE
transposes and halves the x DMA bytes.

Schedule notes (from NTFF traces of earlier revisions):
 - The PE executes in program order, so anything the PE waits on
   (softmax-denominator reciprocal on DVE: 3.3us for a single-partition
   [1,512] row) stalls every later matmul.  The normalize is therefore
   split: the reciprocal runs on ACT as ln -> exp(-x) (both read row 64
   directly, ~0.7us each, and share one activation table set), and the
   PE-side broadcast matmul + DVE multiply are deferred until after the
   NEXT t-block's stage-1 matmuls have been emitted, by which point the
   reciprocal is long done.
 - AV matmuls are emitted one k-tile behind their QK pair so the ACT
   exp of tile kt hides under the QK matmuls of tile kt+1.
 - The AllGather has a ~27us fixed cost regardless of payload; all four
   output projections are deferred to the tail so proj(0..2) hides most
   of AllGather(3).
 - PSUM: scores/stage1/proj accumulators share one [P,2,512] tag
   (2 banks x 2 bufs) and the four per-head y_aug tiles get their own
   bank each (4 x 1), totalling exactly 8 banks.

Per-core dataflow, pipelined over 512-row t-blocks:
  stage 1: qk^T[., tb] = W_qk^T @ x^T (per-partition bias on the PSUM
           copyback, bf16 out); V[tb] = x @ W_v (bias via ones-row
           matmul into the accumulating PSUM), stored per 128-row
           k-tile with an appended ones column so the AV matmul also
           emits the softmax denominator for free.
  stage 2 (q block qb=tb): per head pair (even/odd heads on partition
           halves of the same qkT chunk): both heads' scores into one
           2-bank PSUM tile -> one exp(0.125*s) on ACT -> one
           broadcast-masked multiply (diagonal tiles only) -> per head
           y_aug^T[65,512] += V_aug^T @ e^T (row 64 = sum of exp).
  stage 3: AllGather y^T slice (bf16) across the 4-core group, then
           out[:, col slice] = y^T.T @ W_p slice + bias.
"""

import numpy as np

import concourse.bass as bass  # noqa: F401  (bass types via tc)
import concourse.mybir as mybir
import concourse.tile as tile
from concourse import bacc

P = 128
B, T_FULL, C, H, D = 2, 2048, 1024, 16, 64
F32 = mybir.dt.float32
BF16 = mybir.dt.bfloat16
EXP = mybir.ActivationFunctionType.Exp
LN = mybir.ActivationFunctionType.Ln
BYPASS = mybir.AluOpType.bypass


class Cfg:
    def __init__(self, n_cores, group_size, T, repeat=1, **_compat):
        self.repeat = repeat
        self.n_cores = n_cores
        self.GS = group_size               # cores per batch group
        self.T = T                         # sequence length per core
        self.HL = H // group_size          # heads per core
        assert self.HL % 2 == 0
        self.NP = C // group_size          # c_proj columns per core
        self.CC = C // P                   # contraction chunks (8)
        self.TB = T // 512                 # t-blocks == q blocks
        self.QB = T // 512
        self.KT = T // P                   # k tiles
        self.QKCH = self.HL                # qk^T partition chunks (Q | K)
        self.VW = 68                       # per-head V stride: 64 V + 1 ones
        if n_cores == 8:
            self.replica_groups = [[0, 1, 2, 3], [4, 5, 6, 7]]
        elif n_cores == 4:
            self.replica_groups = [[0, 1], [2, 3]]
        elif n_cores == 1:
            self.replica_groups = [[0]]
        else:
            raise ValueError(n_cores)


CFG_FULL = Cfg(8, 4, T_FULL)


def _patch_act_tables():
    """Make Exp and Ln resolve to the one table set containing both.

    bacc's insert_act_table_loads picks, per activation, the first
    act_info.json set containing its function: Exp -> exp_and_others,
    Ln -> natural_log.  A kernel using both then reloads tables
    (~1.3us each, on ACT) at every normalize.  Restricting Exp/Ln to
    natural_log_exp_and_others (set indices preserved) yields a single
    table load for the whole kernel.
    """
    import concourse.bacc as bacc_mod
    import concourse.hw_specs as hw_specs
    if getattr(bacc_mod, "_mha_act_tables_patched", False):
        return
    orig = hw_specs.get_activation_tables
    keep = "natural_log_exp_and_others"

    def patched(arch):
        t = {k: set(v) for k, v in orig(arch).items()}
        if keep in t and EXP in t[keep] and LN in t[keep]:
            for name in t:
                if name != keep:
                    t[name] -= {EXP, LN}
        return t

    bacc_mod.get_activation_tables = patched
    bacc_mod._mha_act_tables_patched = True


def emit(tc, outs, ins, cfg):
    """Emit the SPMD program. outs/ins are dicts of DRAM APs."""
    for rep in range(cfg.repeat):
        _emit_once(tc, outs["out"], ins, cfg, rep)


def _emit_once(tc, out, ins, cfg, rep):
    nc = tc.nc
    GS, T, HL, NP, CC, VW = cfg.GS, cfg.T, cfg.HL, cfg.NP, cfg.CC, cfg.VW
    QKCH = cfg.QKCH

    xT = ins["xT"]            # [P, TB, CC, 512] bf16 (host-transposed x,
                              # t-block-major SBUF layout)
    wqk = ins["wqk"]          # [P, QKCH, CC, 128] bf16 (SBUF layout, m-major)
    wv = ins["wv"]            # [P, CC, HL*64] bf16 (SBUF layout)
    bqk = ins["bqk"]          # [P, QKCH] f32 (chunk-major per-partition bias)
    wp = ins["wp"]            # [P, CC, NP] bf16 (SBUF layout)
    bp = ins["bp"]            # [1, NP] bf16 (c_proj bias + folded V bias)
    masks = ins["masks"]      # [P, 4, 512] bf16

    from contextlib import ExitStack
    with ExitStack() as _stk:
        persist = _stk.enter_context(tc.tile_pool(name="persist", bufs=1))
        s2 = _stk.enter_context(tc.tile_pool(name="s2", bufs=4))
        s3 = _stk.enter_context(tc.tile_pool(name="s3", bufs=2))
        dram = _stk.enter_context(
            tc.tile_pool(name="dram", bufs=1, space="DRAM"))
        # PSUM: "s" [P,2,512] x2 = 4 banks shared by scores + all f32
        # accumulators; "y" [65,512] x4 = 4 banks.  Total 8.
        ps = _stk.enter_context(tc.tile_pool(
            name="ps", bufs=2, space="PSUM"))
        ps_y = _stk.enter_context(tc.tile_pool(
            name="ps_y", bufs=4, space="PSUM"))

        def acc_tile(name):
            # 1-bank accumulator carved out of a 2-bank "s" slot
            t = ps.tile([P, 2, 512], F32, tag="s", name=name)
            return t[:, 0, :]

        # ---- persistent SBUF tensors ----
        xT_all = persist.tile([P, cfg.TB, CC, 512], BF16, tag="xT")
        qkT = persist.tile([P, QKCH, T], BF16, tag="qkT")
        vsb = persist.tile([P, cfg.KT, HL * VW], BF16, tag="vsb")
        mask_sb = persist.tile([P, 4, 512], BF16, tag="mask")
        ones_row = persist.tile([1, P], BF16, tag="ones_row")
        ones65 = persist.tile([65, 64], BF16, tag="ones65")
        wp_sb = persist.tile([P, CC, NP], BF16, tag="wp")
        bp_sb = persist.tile([1, NP], BF16, tag="bp")
        wqk_sb = persist.tile([P, QKCH, CC, P], BF16, tag="wqk")
        wv_sb = persist.tile([P, CC, HL * D], BF16, tag="wv")
        bqk_sb = persist.tile([P, QKCH], F32, tag="bqk")
        ag_sb = [persist.tile([P, CC, 512], BF16, tag=f"ag{qb}",
                              name=f"ag_sb{qb}_{rep}")
                 for qb in range(cfg.QB)]

        # All weight tensors arrive in their SBUF layout (contiguous
        # per-partition lines; see make_core_inputs).  Only the bytes
        # stage1(0) needs go first — x^T block 0 per cc-chunk on the
        # sync ring (subtile deps let the cc-loop chase the DMA) racing
        # just wqk/bqk on the gpsimd ring; everything else is ordered
        # behind them on the same two rings by need time (masks at
        # attention(0) ~25us, wv mid-stage1, x^T rest at stage1(1)).
        nc.sync.dma_start(xT_all[:, 0, 0:CC // 2], xT[:, 0, 0:CC // 2])
        nc.sync.dma_start(xT_all[:, 0, CC // 2:], xT[:, 0, CC // 2:])
        nc.gpsimd.dma_start(wqk_sb[:, 0], wqk[:, 0])
        nc.gpsimd.dma_start(bqk_sb[:], bqk)
        for m in range(1, QKCH):
            nc.gpsimd.dma_start(wqk_sb[:, m], wqk[:, m])
        nc.sync.dma_start(mask_sb[:], masks)
        nc.gpsimd.dma_start(wv_sb[:], wv)
        nc.sync.dma_start(xT_all[:, 1:], xT[:, 1:])
        nc.gpsimd.dma_start(wp_sb[:], wp)
        nc.gpsimd.dma_start(bp_sb[:], bp)

        nc.vector.memset(ones_row[:], 1.0)
        nc.vector.memset(ones65[64:65, :], 1.0)
        vsb_h = vsb.rearrange("p k (h w) -> p k h w", w=VW)
        nc.vector.memset(vsb_h[:, :, :, 64:65], 1.0)

        ag_in = [
            dram.tile([HL * D, 512], BF16, tag=f"agin{qb}",
                      name=f"agin{qb}_{rep}")
            for qb in range(cfg.QB)
        ]
        ag_out = [
            dram.tile([GS * HL * D, 512], BF16, tag=f"agout{qb}",
                      name=f"agout{qb}_{rep}")
            for qb in range(cfg.QB)
        ]


        def stage1_qk(tb):
            # qk^T: lhsT = W chunk (stationary), rhs = x^T chunk
            for m in range(QKCH):
                acc = acc_tile(f"acc{tb}_{m}_{rep}")
                for cc in range(CC):
                    nc.tensor.matmul(
                        acc,
                        wqk_sb[:, m, cc, :],
                        xT_all[:, tb, cc, :],
                        start=(cc == 0),
                        stop=(cc == CC - 1),
                    )
                nc.vector.tensor_scalar_add(
                    qkT[:, m, tb * 512:(tb + 1) * 512], acc,
                    bqk_sb[:, m:m + 1])

        def stage1_v(tb):
            # V natural: lhsT = x^T chunk (stationary), rhs = W_v.
            # No bias matmul: bv is constant per (head, d), so it passes
            # through the softmax average unchanged and is folded into
            # the c_proj bias on the host.
            for ts in range(4):
                kt = tb * 4 + ts
                vp = acc_tile(f"vp{kt}_{rep}")
                for cc in range(CC):
                    nc.tensor.matmul(
                        vp[:, 0:HL * D],
                        xT_all[:, tb, cc, ts * P:(ts + 1) * P],
                        wv_sb[:, cc, :],
                        start=(cc == 0),
                        stop=(cc == CC - 1),
                    )
                nc.vector.tensor_copy(
                    vsb_h[:, kt, :, 0:64],
                    vp[:, 0:HL * D].rearrange("p (h d) -> p h d", d=D),
                )

        def norm_pre(qb, h, y):
            # softmax denominator reciprocal on ACT: rec = exp(-ln d).
            # Both funcs live in the natural_log_exp table set, and ACT
            # reads the PSUM row directly — no single-partition DVE op.
            lnd = s2.tile([65, 512], F32, tag="lnd", bufs=2,
                          name=f"lnd{qb}_{h}_{rep}")
            nc.scalar.activation(lnd[64:65, :], y[64:65, :], LN)
            rec = s2.tile([65, 512], BF16, tag="rec", bufs=4,
                          name=f"rec{qb}_{h}_{rep}")
            nc.scalar.activation(rec[64:65, :], lnd[64:65, :], EXP,
                                 scale=-1.0)
            return rec

        def norm_post(qb, h, y, rec, copy_on_act=True):
            bc = acc_tile(f"bc{qb}_{h}_{rep}")
            nc.tensor.matmul(
                bc[0:64, :], ones65[64:65, :], rec[64:65, :],
                start=True, stop=True,
            )
            bc_sb = s2.tile([64, 512], F32, tag="bc_sb", bufs=2,
                            name=f"bcs{qb}_{h}_{rep}")
            if copy_on_act:
                # mid-pipeline: keep DVE free for the yn muls that gate
                # the next attention block's y-bank reuse (ACT has slack
                # here and reads PSUM directly)
                nc.scalar.activation(
                    bc_sb[:], bc[0:64, :],
                    mybir.ActivationFunctionType.Copy)
            else:
                # tail: ACT is the longer pole (ln/exp chain just ran),
                # DVE is idle
                nc.vector.tensor_copy(bc_sb[:], bc[0:64, :])
            yn = s2.tile([64, 512], BF16, tag="yn", bufs=4,
                         name=f"yn{qb}_{h}_{rep}")
            nc.vector.tensor_mul(yn[:], y[0:64, :], bc_sb[:])
            nc.sync.dma_start(ag_in[qb][h * 64:(h + 1) * 64, :], yn[:])

        def _kt_order(qb):
            # one unmasked off-diagonal tile first (its AV has no DVE
            # mask dependency, and the flush/stage1 DVE backlog at the
            # block boundary hasn't drained yet), then the diagonal
            # tiles (longest exp->mask->AV chains), then the rest.
            nkt = 4 * qb + 4
            return (list(range(min(1, qb))) +
                    list(range(4 * qb, nkt)) +
                    list(range(min(1, qb), 4 * qb)))

        def attention_pairs(qb, hps):
            # The head pairs' kt loops interleaved: the PE alternates
            # pairs' QK matmuls (each pair's even/odd heads overlap on
            # disjoint row strips), AV pairs are emitted one k-tile
            # behind, so every exp hides under independent QK work.
            nkt = 4 * qb + 4
            kt_order = _kt_order(qb)
            pbs = [slice(0, 64), slice(64, 128)]
            ys = {hp: [ps_y.tile([65, 512], F32, tag="y",
                                 name=f"y{qb}_{2 * hp + i}_{rep}")
                       for i in range(2)]
                  for hp in hps}
            # AV emission runs AV_DEPTH k-tiles behind QK: the first
            # AVs land ~2.5us into the block, past both the exp latency
            # and the boundary DVE backlog (yn muls releasing y banks).
            AV_DEPTH = 2
            av_pend = []
            for ki, kt in enumerate(kt_order):
                j = kt - 4 * qb
                lo = 128 * j if j > 0 else 0
                es = []
                for hp in hps:
                    qch, kch = hp, QKCH // 2 + hp
                    s = ps.tile([P, 2, 512], F32, tag="s",
                                name=f"s{qb}_{kt}_{hp}_{rep}")
                    for i in range(2):
                        nc.tensor.matmul(
                            s[:, i, lo:],
                            qkT[pbs[i], kch, kt * P:(kt + 1) * P],
                            qkT[pbs[i], qch, qb * 512 + lo:(qb + 1) * 512],
                            start=True, stop=True,
                        )
                    es.append(s)
                if len(av_pend) >= AV_DEPTH:
                    for prev in av_pend.pop(0):
                        prev()
                av_prev = []
                for hx, hp in enumerate(hps):
                    e = s2.tile([P, 2, 512], BF16, tag="e", bufs=6,
                                name=f"e{qb}_{kt}_{hp}_{rep}")
                    nc.scalar.activation(
                        e[:, :, lo:], es[hx][:, :, lo:], EXP, scale=0.125)
                    if j >= 0:
                        nc.vector.tensor_mul(
                            e[:, :, lo:], e[:, :, lo:],
                            mask_sb[:, j:j + 1, lo:].to_broadcast(
                                [P, 2, 512 - lo]))

                    def av(ki=ki, lo=lo, kt=kt, e=e, hp=hp):
                        for i in range(2):
                            nc.tensor.matmul(
                                ys[hp][i][:, lo:],
                                vsb[:, kt,
                                    (2 * hp + i) * VW:(2 * hp + i) * VW + 65],
                                e[:, i, lo:],
                                start=(ki == 0), stop=(ki == nkt - 1),
                            )
                    av_prev.append(av)
                av_pend.append(av_prev)
            for group in av_pend:
                for prev in group:
                    prev()
            posts = []
            for hp in hps:
                for i in range(2):
                    h = 2 * hp + i
                    posts.append((qb, h, ys[hp][i],
                                  norm_pre(qb, h, ys[hp][i])))
            return posts

        def attention(qb):
            return attention_pairs(qb, list(range(HL // 2)))

        def flush(posts, copy_on_act=True):
            for qb, h, y, rec in posts:
                norm_post(qb, h, y, rec, copy_on_act)

        def allgather(qb):
            nc.gpsimd.collective_compute(
                "AllGather", BYPASS,
                replica_groups=cfg.replica_groups,
                ins=[ag_in[qb].opt()],
                outs=[ag_out[qb].opt()],
            )
            # prefetch the gathered y^T into SBUF on the gpsimd ring:
            # its wait-for-collective would stall any other engine's
            # pipeline (a DMA_DIRECT2D sem-wait blocks the issuing
            # engine), but gpsimd only runs the collectives, which
            # serialize on the CC cores anyway.  The last block is on
            # the critical tail: slice it per tt so proj(3)'s first
            # matmuls chase the DMA.
            ag_r = ag_out[qb].rearrange("(c p) t -> p c t", p=P)
            if qb < cfg.QB - 1:
                nc.gpsimd.dma_start(ag_sb[qb][:], ag_r[:])
            else:
                for tt in range(4):
                    tsl = slice(tt * P, (tt + 1) * P)
                    nc.gpsimd.dma_start(ag_sb[qb][:, :, tsl],
                                        ag_r[:, :, tsl])

        def proj(qb):
            for tt in range(4):
                tsl = slice(tt * P, (tt + 1) * P)
                op = acc_tile(f"op{qb}_{tt}_{rep}")
                for cc in range(CC):
                    nc.tensor.matmul(
                        op[:, 0:NP], ag_sb[qb][:, cc, tsl],
                        wp_sb[:, cc, :], start=(cc == 0), stop=False,
                    )
                nc.tensor.matmul(
                    op[:, 0:NP], ones_row[:1, :], bp_sb[:1, :],
                    start=False, stop=True,
                )
                o_sb = s3.tile([P, NP], F32, tag="osb", bufs=4)
                nc.vector.tensor_copy(o_sb[:], op[:, 0:NP])
                row = (qb * 4 + tt) * P
                nc.sync.dma_start(out[row:row + P, :], o_sb[:])

        # pipeline: attention(qb) consumes exactly the k-tiles stage1(tb)
        # produced; normalize posts flush between the next stage1's QK
        # and V parts — late enough that the ACT reciprocal chain is
        # done (no PE wait on the bc matmuls), early enough that the
        # yn multiplies clear DVE before the next attention's first AV
        # recycles the y banks.  Projs all run in the tail where
        # proj(0..2) hides AllGather(3)'s fixed cost.
        stage1_qk(0)
        stage1_v(0)
        for tb in range(cfg.TB):
            posts = attention(tb)
            if tb + 1 < cfg.TB:
                stage1_qk(tb + 1)
                flush(posts)
                allgather(tb)
                stage1_v(tb + 1)
            else:
                flush(posts, copy_on_act=False)
                allgather(tb)
        for qb in range(cfg.QB):
            proj(qb)


def make_core_inputs(x_full, c_attn_w, c_attn_b, c_proj_w, c_proj_b, cfg,
                     core):
    """Host-side input sharding (and bf16 cast) for one core."""
    import ml_dtypes
    bf16 = ml_dtypes.bfloat16

    GS, HL, NP, T = cfg.GS, cfg.HL, cfg.NP, cfg.T
    g, rk = divmod(core, GS)
    g = g % B  # tolerate more groups than batches (sim configs)
    hs = slice(rk * HL * D, (rk + 1) * HL * D)
    wq = c_attn_w[:, 0 * C:1 * C][:, hs]
    wk = c_attn_w[:, 1 * C:2 * C][:, hs]
    wv = c_attn_w[:, 2 * C:3 * C][:, hs]
    bq = c_attn_b[0 * C:1 * C][hs]
    bk = c_attn_b[1 * C:2 * C][hs]
    bv = c_attn_b[2 * C:3 * C][hs]
    cs = slice(rk * NP, (rk + 1) * NP)

    pp = np.arange(P)[:, None, None]
    jj = np.arange(4)[None, :, None]
    qq = np.arange(512)[None, None, :]
    masks = (qq >= pp + 128 * jj)

    CCn = cfg.CC

    def sbuf_layout(w):
        # [C, N] -> [P, CC, N]: partition-major chunked contraction
        return np.ascontiguousarray(
            w.reshape(CCn, P, w.shape[1]).transpose(1, 0, 2))

    # x^T in t-block-major SBUF layout [P, TB, CC, 512]: every DMA line
    # is per-partition contiguous (8 KB for the bulk transfer)
    xT_pre = np.ascontiguousarray(
        x_full[g, :T].T.astype(bf16)          # [C, T]
        .reshape(CCn, P, T // 512, 512)       # [cc, p, tb, t']
        .transpose(1, 2, 0, 3))               # [p, tb, cc, t']

    wqk_cn = np.concatenate([wq, wk], axis=1).astype(bf16)  # [C, QKCH*128]
    # [C, M*128] -> [P, M, CC, 128] (m-major so each m-slice is one
    # contiguous 2KB-per-partition DMA)
    wqk_pre = np.ascontiguousarray(
        wqk_cn.reshape(CCn, P, cfg.QKCH, P).transpose(1, 2, 0, 3))

    # bv is constant per (head, d): it passes through the softmax
    # average unchanged, so its contribution to the output is exactly
    # bv_full @ Wp — fold it into the c_proj bias here.
    bv_full = c_attn_b[2 * C:3 * C]
    bp_folded = bv_full @ c_proj_w[:, cs] + c_proj_b[cs]

    return {
        "xT": xT_pre,
        "wqk": wqk_pre,
        "wv": sbuf_layout(wv.astype(bf16)),
        "bqk": np.ascontiguousarray(
            np.concatenate([bq, bk]).reshape(cfg.QKCH, P).T, np.float32),
        "wp": sbuf_layout(c_proj_w[:, cs].astype(bf16)),
        "bp": np.ascontiguousarray(bp_folded[None, :].astype(bf16)),
        "masks": masks.astype(bf16),
    }


_CACHE = {}


def _input_shapes_dtypes(cfg):
    return {
        "xT": ((P, cfg.TB, cfg.CC, 512), BF16),
        "wqk": ((P, cfg.QKCH, cfg.CC, P), BF16),
        "wv": ((P, cfg.CC, cfg.HL * D), BF16),
        "bqk": ((P, cfg.QKCH), F32),
        "wp": ((P, cfg.CC, cfg.NP), BF16),
        "bp": ((1, cfg.NP), BF16),
        "masks": ((P, 4, 512), BF16),
    }


def _build_full():
    if "nc" in _CACHE:
        return _CACHE["nc"]
    _patch_act_tables()
    cfg = CFG_FULL
    nc = bacc.Bacc(
        "TRN2", target_bir_lowering=False, debug=False,
        num_devices=cfg.n_cores,
    )
    ins = {}
    for name, (shape, dt) in _input_shapes_dtypes(cfg).items():
        ins[name] = nc.dram_tensor(
            name, list(shape), dt, kind="ExternalInput").ap()
    outs = {
        "out": nc.dram_tensor(
            "out", [cfg.T, cfg.NP], F32, kind="ExternalOutput").ap()
    }
    with tile.TileContext(nc) as tc:
        emit(tc, outs, ins, cfg)
    nc.compile()
    _CACHE["nc"] = nc
    return nc


def kernel(**inputs):
    from concourse.bass_utils import run_bass_kernel_spmd

    cfg = CFG_FULL
    x = np.asarray(inputs["x"], np.float32)
    c_attn_w = np.asarray(inputs["c_attn_w"], np.float32)
    c_attn_b = np.asarray(inputs["c_attn_b"], np.float32)
    c_proj_w = np.asarray(inputs["c_proj_w"], np.float32)
    c_proj_b = np.asarray(inputs["c_proj_b"], np.float32)

    nc = _build_full()
    in_maps = [
        make_core_inputs(x, c_attn_w, c_attn_b, c_proj_w, c_proj_b, cfg, core)
        for core in range(cfg.n_cores)
    ]
    res = run_bass_kernel_spmd(nc, in_maps, core_ids=list(range(cfg.n_cores)))
    out = np.empty((B, T_FULL, C), np.float32)
    for core in range(cfg.n_cores):
        g, rk = divmod(core, cfg.GS)
        out[g, :, rk * cfg.NP:(rk + 1) * cfg.NP] = res.results[core]["out"]
    return out
